# revision 55
# baseline (speedup 1.0000x reference)
"""Trainium2 Bass kernel for nn_BatchHighOrderActivation.

Reference semantics (per batch b, channel g):
    sort the ARITY=4 values x = X[b,g,:], build barycentric coefficients from
    the sorted gaps, gather params rows by reverse-cumsum bitmasks, contract.

Sort/gather-free reformulation (multilinear simplex / Lovasz form):
    out[b,g,:] = sum_{m=0..15} relu(w[b,g,m]) * params'[g,m,:]
    w[m]  = min_{i in m} x_i - max_{i not in m} x_i     for m in 1..14
    w[15] = min_i x_i,  w[0] = -min_i x_i               (x = relu(x)-relu(-x))
    params'[g,0,:] = -params[g,15,:] so the m0/m15 pair reproduces the
    un-relu'd min_i x_i * params[g,15,:] term; every column is then relu'd
    uniformly, which lets the relu ride the PSUM->SBUF evacuation for free.

Kernel structure per core (pure batch data-parallel sharding, 512 rows/core),
fp16 internal compute AND fp16 output wire format (host upconverts):
  - host: X de-interleaved to fp16 arity-planes, two 128-row b-tiles packed
          per plane; params expanded to an fp16 block-diagonal table
          (8 channels/group, K-order (m,gl)) with row m=0 := -row m=15
  - DVE : subset min/max tree at FD=1024 (b-tile pairs, (0,1) ops first so
          they start on the first half-load), then the qmin/neg/14-sub ops
          into a per-PAIR W tile [128, (t,q,m,gl)].  The q-half sub blocks
          are PAIRED across the two b-tiles (3-free-dim [128,(2,32,8)] APs
          at full fp16-2x rate) except bt0-hf0/bt1-hf0 which stay solo to
          keep the head short and the b-tile-1 pipeline fed
  - PE  : transpose W 128x128 chunks via fp16 identity matmul, SOFTWARE
          PIPELINED one group ahead of the block matmuls
  - DVE : W^T evacuation with FUSED RELU (tensor_scalar_max, fp16 2x mode)
  - PE  : block-diagonal fp16 matmul (K=(m,gl)=128, N=8ch*32=256), fp32 PSUM
  - DVE/ACT: [128,1024] PSUM->SBUF output evacuations casting fp32->fp16:
          ACT, except every 5th on the tree/sub-free last b-tile on DVE, and
          the very last group split per-half across BOTH engines
  - X pair0 load split by arity-plane pairs across the sync + scalar HWDGE
    rings; pair1's load issued up-front on sync (never queues behind
    stores); params ride the scalar ring behind pair0's second half;
    mid-kernel stores alternate sync/gpsimd rings, the last b-tile stores
    at [128,2048] grain on sync and the final two at [128,1024]

Measured on 8-core SPMD axon trn2: ~101-103us typical (vs 108-109us staged
baseline).  DVE is the binding engine (~78us busy, ~98% occupancy from
first tree op to last evac); ACT carries ~65us of output evacuation with
~2us slack; the chip appears to down-throttle all engine clocks ~20% when
extra concurrent engine activity (SWDGE during compute, early ACT work) is
added, so rebalancing work onto idle engines loses — only total-work
reductions and head/tail trims win.  The mid-window steady state is
ACT-evac-paced through PSUM buffer recycling (~1.14us per [128,1024] pm,
PE shows matching ~0.4us gaps per cycle); a 2-group PE skew and denser
DVE casts were both measured neutral-to-worse, and the DVE/ACT evac split
sits at its LP optimum, so the remaining floor is the evacuation itself.
NOTE on benchmarking: the ~20% all-engine slowdown episodes (runs landing
~109-123us with every op uniformly slower) are a TRANSIENT DEVICE throttle
state — a byte-identical verified binary measured ~121us during one episode
and ~103us after a ~3min idle cooldown.  Treat any slow sample as suspect
until reproduced after idle time; don't attribute it to a code change.
"""

import numpy as np
from contextlib import ExitStack

import concourse.bass as bass
import concourse.mybir as mybir
import concourse.tile as tile
from concourse import bacc
from concourse.bass_utils import run_bass_kernel_spmd
from concourse.masks import make_identity

F32 = mybir.dt.float32
F16 = mybir.dt.float16
NCORES = 8
B, G, A, O = 4096, 512, 4, 32
BS = B // NCORES        # 512 batch rows per core
NBT = BS // 128         # 4 b-tiles per core
NPAIR = NBT // 2        # b-tile pairs (tree computed at FD=2*G)
NQ = G // 8             # 64 channel groups of 8

# (0,1) first: its two tree ops only need arity planes 0/1, which arrive in
# the first half-load of the split pair0 DMA
_PAIRS = [(0, 1), (0, 2), (0, 3), (1, 2), (1, 3), (2, 3)]
_TRIPLES = [(0, 1, 2), (0, 1, 3), (0, 2, 3), (1, 2, 3)]
_SUBS = [3, 5, 9, 6, 10, 12, 7, 11, 13, 14, 1, 2, 4, 8]

_cached_nc = None


def _build_program():
    nc = bacc.Bacc("TRN2", target_bir_lowering=False, debug=False, num_devices=NCORES)

    # X pre-deinterleaved on host into fp16 planes, b-tile pairs packed
    # per row: row (pr*128+p) holds [a, t, g] for batch rows pr*256+t*128+p
    x_d = nc.dram_tensor("x", [NPAIR * 128, A * 2 * G], F16, kind="ExternalInput").ap()
    pbd_d = nc.dram_tensor("pbd", [128, NQ * 256], F16, kind="ExternalInput").ap()
    out_d = nc.dram_tensor("out", [BS, G * O], F16, kind="ExternalOutput").ap()

    with ExitStack() as ctx:
        tc = ctx.enter_context(tile.TileContext(nc))
        persist = ctx.enter_context(tc.tile_pool(name="persist", bufs=1))
        plpool = ctx.enter_context(tc.tile_pool(name="pl", bufs=2))
        treep = ctx.enter_context(tc.tile_pool(name="tree", bufs=2))
        wpool = ctx.enter_context(tc.tile_pool(name="w", bufs=1))
        lhsp = ctx.enter_context(tc.tile_pool(name="lt", bufs=3))
        stgp = ctx.enter_context(tc.tile_pool(name="stg", bufs=2))
        ptp = ctx.enter_context(tc.tile_pool(name="pt", bufs=2, space="PSUM"))
        pmp = ctx.enter_context(tc.tile_pool(name="pm", bufs=3, space="PSUM"))

        # b-tile pairs: the 1024-wide tree ops amortize DVE's per-op overhead
        groups = [[0, 1], [2, 3]]

        pbd = [
            persist.tile([128, 16 * 256], F16, name=f"pbd{i}") for i in range(4)
        ]
        identity = persist.tile([128, 128], F16)
        # pair0's X load split by arity-plane pairs across two HWDGE rings:
        # both halves transfer concurrently, so the tree's first op (planes
        # 0/1, reordered first) starts ~2-3us earlier than one full-row load
        pl0 = plpool.tile([128, A, 2, G], F16, tag="pl")
        nc.sync.dma_start(
            pl0[:, 0:2, :, :].rearrange("p a t g -> p (a t g)"),
            x_d[0:128, 0:2 * 2 * G],
        )
        nc.scalar.dma_start(
            pl0[:, 2:4, :, :].rearrange("p a t g -> p (a t g)"),
            x_d[0:128, 2 * 2 * G:4 * 2 * G],
        )
        # pair1's load issued up-front too (sync ring, behind pl0's first
        # half) so it never queues behind bt0/bt1's output stores
        pl1 = plpool.tile([128, A, 2, G], F16, tag="pl", name="pl1")
        nc.sync.dma_start(
            pl1[:].rearrange("p a t g -> p (a t g)"),
            x_d[128:256, :],
        )
        for i in range(4):
            nc.scalar.dma_start(pbd[i][:], pbd_d[:, i * 4096:(i + 1) * 4096])
        make_identity(nc, identity[:])

        ev = [0]
        oev = [0]
        stc = [0]
        pend = [None]
        stgs = {}

        def store(dst_ap, src_ap):
            # alternate stores between the sync and gpsimd rings so the
            # store stream isn't serialized on one queue and the final
            # store rides an empty ring
            eng = nc.sync if stc[0] % 2 == 0 else nc.gpsimd
            stc[0] += 1
            eng.dma_start(dst_ap, src_ap)

        def flush_group(final=False):
            """Emit the matmuls + output evacuation + stores for the group
            held in ``pend`` (deferred by one group for the PE skew)."""
            if pend[0] is None:
                return
            fbt, fhf, fgp, fgqi, fq0, flt, flast = pend[0]
            pend[0] = None
            stg = stgs[(fbt, fhf)]
            for half in range(2):
                pm = pmp.tile([128, 1024], F32, tag="pm")
                for j2 in range(4):
                    j = half * 4 + j2
                    qq = fq0 + j
                    nc.tensor.matmul(
                        pm[:, j2 * 256:(j2 + 1) * 256],
                        flt[:, j * 128:(j + 1) * 128],
                        pbd[qq // 16][:, (qq % 16) * 256:(qq % 16 + 1) * 256],
                        start=True,
                        stop=True,
                    )
                dst = stg[:, fgp * 4096 + fgqi * 2048 + half * 1024:
                          fgp * 4096 + fgqi * 2048 + (half + 1) * 1024]
                if final:
                    # very last group: halve the evac latency by running
                    # DVE and ACT on half-chunks concurrently
                    nc.vector.tensor_copy(dst[:, 0:512], pm[:, 0:512])
                    nc.scalar.copy(dst[:, 512:1024], pm[:, 512:1024])
                else:
                    # out evacuation fp32->fp16: ACT, except a few on DVE on
                    # the tree-free last b-tile to balance the engine load
                    dve_out = (oev[0] % 5 == 2) if flast else False
                    if dve_out:
                        nc.vector.tensor_copy(dst, pm[:])
                    else:
                        nc.scalar.copy(dst, pm[:])
                oev[0] += 1
            if flast:
                # finest stores at the drain tail: one per (gp, gqi), all
                # on the (by now empty) sync HWDGE ring; the final group
                # stores per-half so the last transfer is only 256KB
                qq0 = fhf * 32 + fgp * 16 + fgqi * 8
                if final:
                    for half in range(2):
                        nc.sync.dma_start(
                            out_d[fbt * 128:(fbt + 1) * 128,
                                  (qq0 + half * 4) * 256:(qq0 + half * 4 + 4) * 256],
                            stg[:, fgp * 4096 + fgqi * 2048 + half * 1024:
                                fgp * 4096 + fgqi * 2048 + (half + 1) * 1024],
                        )
                else:
                    nc.sync.dma_start(
                        out_d[fbt * 128:(fbt + 1) * 128,
                              qq0 * 256:(qq0 + 8) * 256],
                        stg[:, fgp * 4096 + fgqi * 2048:
                            fgp * 4096 + (fgqi + 1) * 2048],
                    )
            elif fgqi == 1:
                # per-gp stores [128,4096] (1MB) alternating rings
                store(
                    out_d[fbt * 128:(fbt + 1) * 128,
                          (fhf * 32 + fgp * 16) * 256:
                          (fhf * 32 + fgp * 16 + 16) * 256],
                    stg[:, fgp * 4096:(fgp + 1) * 4096],
                )

        for gi, grp in enumerate(groups):
            gw = len(grp)
            pl = pl0 if gi == 0 else pl1
            # group-wide arity planes [128, gw*G]
            s2 = [pl[:, i, :, :] for i in range(A)]

            tr = treep.tile([128, 20, gw, G], F16, tag="tree", name=f"tr{gi}")
            slot = [0]
            mn, mx = {}, {}

            def alloc():
                ap = tr[:, slot[0], :, :]
                slot[0] += 1
                return ap

            # (0,1) min+max first: they only need the first half-load
            mn[(0, 1)] = alloc()
            nc.vector.tensor_tensor(mn[(0, 1)], s2[0], s2[1], mybir.AluOpType.min)
            mx[(0, 1)] = alloc()
            nc.vector.tensor_tensor(mx[(0, 1)], s2[0], s2[1], mybir.AluOpType.max)
            for (i, j) in _PAIRS[1:]:
                mn[(i, j)] = alloc()
                nc.vector.tensor_tensor(mn[(i, j)], s2[i], s2[j], mybir.AluOpType.min)
            for (i, j) in _PAIRS[1:]:
                mx[(i, j)] = alloc()
                nc.vector.tensor_tensor(mx[(i, j)], s2[i], s2[j], mybir.AluOpType.max)
            for (i, j, k) in _TRIPLES:
                mn[(i, j, k)] = alloc()
                nc.vector.tensor_tensor(mn[(i, j, k)], mn[(i, j)], s2[k], mybir.AluOpType.min)
                mx[(i, j, k)] = alloc()
                nc.vector.tensor_tensor(mx[(i, j, k)], mx[(i, j)], s2[k], mybir.AluOpType.max)

            def sub_ap(S):
                return s2[S[0]] if len(S) == 1 else mn[S]

            def sup_ap(Cm):
                return s2[Cm[0]] if len(Cm) == 1 else mx[Cm]

            # one W tile per PAIR, both b-tiles side by side (t dim); the
            # bufs=1 pool reuses the same SBUF for pair1 (its writes only
            # wait on pair0's long-finished transpose reads).
            # W layout: free = t*8192 + q*128 + m*8 + gl (K-order (m,gl));
            # the walrus BIR verifier requires the transpose stationary AP
            # to have one free dim, so per-q chunks must be contiguous.
            wt = wpool.tile([128, 2 * NQ * 128], F16, tag="w", name=f"w{gi}")
            wv5 = wt.rearrange("p (t q m gl) -> p t q m gl", t=2, m=16, gl=8)

            def emit_subs(ts, hf2):
                """Emit the qmin/neg/14-sub ops for q-half ``hf2`` covering
                the b-tile slice ``ts`` of the pair: solo [128,(32,8)] ops,
                or paired [128,(2,32,8)] ops at half the op count."""
                qh = slice(hf2 * 32, hf2 * 32 + 32)
                gh = slice(hf2 * 256, hf2 * 256 + 256)
                nc.vector.tensor_tensor(
                    wv5[:, ts, qh, 15, :],
                    mn[(0, 1, 2)][:, ts, gh], s2[3][:, ts, gh],
                    mybir.AluOpType.min,
                )
                # m0 column: -qmin (x = relu(x)-relu(-x) paired with the
                # negated m=15 params rows): uniformly relu-able W
                nc.vector.tensor_scalar_mul(
                    wv5[:, ts, qh, 0, :], wv5[:, ts, qh, 15, :], -1.0
                )
                for m in _SUBS:
                    S = tuple(i for i in range(A) if (m >> i) & 1)
                    Cm = tuple(i for i in range(A) if not ((m >> i) & 1))
                    nc.vector.tensor_tensor(
                        wv5[:, ts, qh, m, :],
                        sub_ap(S)[:, ts, gh],
                        sup_ap(Cm)[:, ts, gh],
                        mybir.AluOpType.subtract,
                    )

            # which (bt2, hf) emits subs, and over which t-slice:
            #  pair0: bt0-hf0 solo (keeps the head short), hf1 paired at
            #         bt0's position, bt1-hf0 solo; pair1: fully paired at
            #         bt2's positions (bt3 rides free)
            if gi == 0:
                sub_plan = {(0, 0): slice(0, 1), (0, 1): slice(0, 2),
                            (1, 0): slice(1, 2)}
            else:
                sub_plan = {(0, 0): slice(0, 2), (0, 1): slice(0, 2)}

            for bt2, bt in enumerate(grp):
                last_bt = bt == NBT - 1
                # one-group software-pipeline skew: group g+1's transposes
                # are emitted BEFORE group g's matmuls, so the in-order PE
                # fills the W^T-evacuation latency with transpose work
                for hf in range(2):
                    if (bt2, hf) in sub_plan:
                        emit_subs(sub_plan[(bt2, hf)], hf)
                    for gp in range(2):
                        for gqi in range(2):
                            q0 = hf * 32 + gp * 16 + gqi * 8
                            if (bt, hf) not in stgs:
                                stgs[(bt, hf)] = stgp.tile(
                                    [128, 32 * 256], F16, tag="stg",
                                    name=f"stg{bt}_{hf}",
                                )
                            pt = ptp.tile([128, 8 * 128], F16, tag="pt")
                            for j in range(8):
                                q = q0 + j
                                nc.tensor.transpose(
                                    pt[:, j * 128:(j + 1) * 128],
                                    wt[:, (bt2 * NQ + q) * 128:
                                       (bt2 * NQ + q + 1) * 128],
                                    identity[:],
                                )
                            flush_group()
                            lt = lhsp.tile([128, 8 * 128], F16, tag="lt")
                            # W^T evacuation with fused relu on DVE (fp16 2x
                            # mode, ~1.7x cheaper there than on ACT)
                            nc.vector.tensor_scalar_max(lt[:], pt[:], 0.0)
                            ev[0] += 1
                            pend[0] = (bt, hf, gp, gqi, q0, lt, last_bt)
                            if ev[0] <= 2:
                                # first two groups: flush immediately (no
                                # skew) so the ACT evac stream starts ~1us
                                # earlier; the skew re-establishes after
                                flush_group()

        flush_group(final=True)

    nc.compile()
    return nc


def _get_program():
    global _cached_nc
    if _cached_nc is None:
        _cached_nc = _build_program()
    return _cached_nc


def _make_inputs(X, params):
    X = np.ascontiguousarray(X, dtype=np.float32)
    params = np.ascontiguousarray(params, dtype=np.float32)
    P4 = params.reshape(NQ, 8, 16, O)                 # [q, gl, m, o]
    # block-diag table: pbd[m*8+gl, q*256 + gl*32 + o] = params[8q+gl, m, o]
    # row m=0 carries -params[...,15,:] (pairs with the -qmin W column)
    Pb = np.zeros((16, 8, NQ, 8, O), np.float32)
    for gl in range(8):
        Pb[1:, gl, :, gl, :] = P4[:, gl, 1:, :].transpose(1, 0, 2)
        Pb[0, gl, :, gl, :] = -P4[:, gl, 15, :]
    pbd = np.ascontiguousarray(Pb.reshape(128, NQ * 256).astype(np.float16))
    # de-interleave X to per-arity fp16 planes, packing b-tile pairs:
    # xp[c, pr, p, a, t, g] = X[c*BS + pr*256 + t*128 + p, g, a]
    Xp = (X.reshape(NCORES, NBT // 2, 2, 128, G, A)
            .transpose(0, 1, 3, 5, 2, 4)              # c, pr, p, a, t, g
            .astype(np.float16))
    Xp = np.ascontiguousarray(Xp.reshape(NCORES, NPAIR * 128, A * 2 * G))
    in_maps = [
        {"x": Xp[c], "pbd": pbd}
        for c in range(NCORES)
    ]
    return in_maps


def kernel(X, params):
    nc = _get_program()
    in_maps = _make_inputs(X, params)
    res = run_bass_kernel_spmd(nc, in_maps, list(range(NCORES))).results
    out = np.concatenate(
        [res[c]["out"].astype(np.float32).reshape(BS, G, O) for c in range(NCORES)],
        axis=0,
    )
    return out


def kernel_traced(X, params):
    """Like kernel() but also returns the BassKernelResults (profile info)."""
    nc = _get_program()
    in_maps = _make_inputs(X, params)
    br = run_bass_kernel_spmd(nc, in_maps, list(range(NCORES)), trace=True)
    out = np.concatenate(
        [br.results[c]["out"].astype(np.float32).reshape(BS, G, O)
         for c in range(NCORES)],
        axis=0,
    )
    return out, br


# revision 58
# speedup vs baseline: 1.1831x; 1.1831x over previous
"""Trainium2 Bass kernel for nn_BatchHighOrderActivation.

Reference semantics (per batch b, channel g):
    sort the ARITY=4 values x = X[b,g,:], build barycentric coefficients from
    the sorted gaps, gather params rows by reverse-cumsum bitmasks, contract.

Sort/gather-free reformulation (multilinear simplex / Lovasz form):
    out[b,g,:] = sum_{m=0..15} relu(w[b,g,m]) * params'[g,m,:]
    w[m]  = min_{i in m} x_i - max_{i not in m} x_i     for m in 1..14
    w[15] = min_i x_i,  w[0] = -min_i x_i               (x = relu(x)-relu(-x))
    params'[g,0,:] = -params[g,15,:] so the m0/m15 pair reproduces the
    un-relu'd min_i x_i * params[g,15,:] term; every column is then relu'd
    uniformly, which lets the relu ride the PSUM->SBUF evacuation for free.

Kernel structure per core (pure batch data-parallel sharding, 512 rows/core),
fp16 internal compute AND fp16 output wire format (host upconverts):
  - host: X de-interleaved to fp16 arity-planes, two 128-row b-tiles packed
          per plane; params expanded to an fp16 block-diagonal table
          (8 channels/group, K-order (m,gl)) with row m=0 := -row m=15
  - DVE : subset min/max tree at FD=1024 (b-tile pairs, (0,1) ops first so
          they start on the first half-load), then the qmin/neg/14-sub ops
          into a per-PAIR W tile [128, (t,q,m,gl)].  The q-half sub blocks
          are PAIRED across the two b-tiles (3-free-dim [128,(2,32,8)] APs
          at full fp16-2x rate) except bt0-hf0/bt1-hf0 which stay solo to
          keep the head short and the b-tile-1 pipeline fed
  - PE  : transpose W 128x128 chunks via fp16 identity matmul, SOFTWARE
          PIPELINED one group ahead of the block matmuls
  - DVE : W^T evacuation with FUSED RELU (tensor_scalar_max, fp16 2x mode)
  - PE  : block-diagonal fp16 matmul (K=(m,gl)=128, N=8ch*32=256), fp32 PSUM
  - DVE/ACT: [128,1024] PSUM->SBUF output evacuations casting fp32->fp16:
          ACT, except every 5th on the tree/sub-free last b-tile on DVE, and
          the very last group split per-half across BOTH engines
  - X pair0 load split by arity-plane pairs across the sync + scalar HWDGE
    rings; pair1's load issued up-front on sync (never queues behind
    stores); params ride the scalar ring behind pair0's second half;
    mid-kernel stores alternate sync/gpsimd rings, the last b-tile stores
    at [128,2048] grain on sync and the final two at [128,1024]

Measured on 8-core SPMD axon trn2: ~101-103us typical (vs 108-109us staged
baseline).  DVE is the binding engine (~78us busy, ~98% occupancy from
first tree op to last evac); ACT carries ~65us of output evacuation with
~2us slack; the chip appears to down-throttle all engine clocks ~20% when
extra concurrent engine activity (SWDGE during compute, early ACT work) is
added, so rebalancing work onto idle engines loses — only total-work
reductions and head/tail trims win.  The mid-window steady state is
ACT-evac-paced through PSUM buffer recycling (~1.14us per [128,1024] pm,
PE shows matching ~0.4us gaps per cycle); a 2-group PE skew and denser
DVE casts were both measured neutral-to-worse, and the DVE/ACT evac split
sits at its LP optimum, so the remaining floor is the evacuation itself.
NOTE on benchmarking: the ~20% all-engine slowdown episodes (runs landing
~109-123us with every op uniformly slower) are a TRANSIENT DEVICE throttle
state — a byte-identical verified binary measured ~121us during one episode
and ~103us after a ~3min idle cooldown.  Treat any slow sample as suspect
until reproduced after idle time; don't attribute it to a code change.
Open candidate for a future session: extending the skew-free flush to the
first TWO groups (ev[0] <= 2) — its only measurement landed in a throttle
window (~122us, uninformative); the one-group version cleanly won ~1.7us.
"""

import numpy as np
from contextlib import ExitStack

import concourse.bass as bass
import concourse.mybir as mybir
import concourse.tile as tile
from concourse import bacc
from concourse.bass_utils import run_bass_kernel_spmd
from concourse.masks import make_identity

F32 = mybir.dt.float32
F16 = mybir.dt.float16
NCORES = 8
B, G, A, O = 4096, 512, 4, 32
BS = B // NCORES        # 512 batch rows per core
NBT = BS // 128         # 4 b-tiles per core
NPAIR = NBT // 2        # b-tile pairs (tree computed at FD=2*G)
NQ = G // 8             # 64 channel groups of 8

# (0,1) first: its two tree ops only need arity planes 0/1, which arrive in
# the first half-load of the split pair0 DMA
_PAIRS = [(0, 1), (0, 2), (0, 3), (1, 2), (1, 3), (2, 3)]
_TRIPLES = [(0, 1, 2), (0, 1, 3), (0, 2, 3), (1, 2, 3)]
_SUBS = [3, 5, 9, 6, 10, 12, 7, 11, 13, 14, 1, 2, 4, 8]

_cached_nc = None


def _build_program():
    nc = bacc.Bacc("TRN2", target_bir_lowering=False, debug=False, num_devices=NCORES)

    # X pre-deinterleaved on host into fp16 planes, b-tile pairs packed
    # per row: row (pr*128+p) holds [a, t, g] for batch rows pr*256+t*128+p
    x_d = nc.dram_tensor("x", [NPAIR * 128, A * 2 * G], F16, kind="ExternalInput").ap()
    pbd_d = nc.dram_tensor("pbd", [128, NQ * 256], F16, kind="ExternalInput").ap()
    out_d = nc.dram_tensor("out", [BS, G * O], F16, kind="ExternalOutput").ap()

    with ExitStack() as ctx:
        tc = ctx.enter_context(tile.TileContext(nc))
        persist = ctx.enter_context(tc.tile_pool(name="persist", bufs=1))
        plpool = ctx.enter_context(tc.tile_pool(name="pl", bufs=2))
        treep = ctx.enter_context(tc.tile_pool(name="tree", bufs=2))
        wpool = ctx.enter_context(tc.tile_pool(name="w", bufs=1))
        lhsp = ctx.enter_context(tc.tile_pool(name="lt", bufs=3))
        stgp = ctx.enter_context(tc.tile_pool(name="stg", bufs=2))
        ptp = ctx.enter_context(tc.tile_pool(name="pt", bufs=2, space="PSUM"))
        pmp = ctx.enter_context(tc.tile_pool(name="pm", bufs=3, space="PSUM"))

        # b-tile pairs: the 1024-wide tree ops amortize DVE's per-op overhead
        groups = [[0, 1], [2, 3]]

        pbd = [
            persist.tile([128, 16 * 256], F16, name=f"pbd{i}") for i in range(4)
        ]
        identity = persist.tile([128, 128], F16)
        # pair0's X load split by arity-plane pairs across two HWDGE rings:
        # both halves transfer concurrently, so the tree's first op (planes
        # 0/1, reordered first) starts ~2-3us earlier than one full-row load
        pl0 = plpool.tile([128, A, 2, G], F16, tag="pl")
        nc.sync.dma_start(
            pl0[:, 0:2, :, :].rearrange("p a t g -> p (a t g)"),
            x_d[0:128, 0:2 * 2 * G],
        )
        nc.scalar.dma_start(
            pl0[:, 2:4, :, :].rearrange("p a t g -> p (a t g)"),
            x_d[0:128, 2 * 2 * G:4 * 2 * G],
        )
        # pair1's load issued up-front too (sync ring, behind pl0's first
        # half) so it never queues behind bt0/bt1's output stores
        pl1 = plpool.tile([128, A, 2, G], F16, tag="pl", name="pl1")
        nc.sync.dma_start(
            pl1[:].rearrange("p a t g -> p (a t g)"),
            x_d[128:256, :],
        )
        for i in range(4):
            nc.scalar.dma_start(pbd[i][:], pbd_d[:, i * 4096:(i + 1) * 4096])
        make_identity(nc, identity[:])

        ev = [0]
        oev = [0]
        stc = [0]
        pend = [None]
        stgs = {}

        def store(dst_ap, src_ap):
            # alternate stores between the sync and gpsimd rings so the
            # store stream isn't serialized on one queue and the final
            # store rides an empty ring
            eng = nc.sync if stc[0] % 2 == 0 else nc.gpsimd
            stc[0] += 1
            eng.dma_start(dst_ap, src_ap)

        def flush_group(final=False):
            """Emit the matmuls + output evacuation + stores for the group
            held in ``pend`` (deferred by one group for the PE skew)."""
            if pend[0] is None:
                return
            fbt, fhf, fgp, fgqi, fq0, flt, flast = pend[0]
            pend[0] = None
            stg = stgs[(fbt, fhf)]
            for half in range(2):
                pm = pmp.tile([128, 1024], F32, tag="pm")
                for j2 in range(4):
                    j = half * 4 + j2
                    qq = fq0 + j
                    nc.tensor.matmul(
                        pm[:, j2 * 256:(j2 + 1) * 256],
                        flt[:, j * 128:(j + 1) * 128],
                        pbd[qq // 16][:, (qq % 16) * 256:(qq % 16 + 1) * 256],
                        start=True,
                        stop=True,
                    )
                dst = stg[:, fgp * 4096 + fgqi * 2048 + half * 1024:
                          fgp * 4096 + fgqi * 2048 + (half + 1) * 1024]
                if final:
                    # very last group: halve the evac latency by running
                    # DVE and ACT on half-chunks concurrently
                    nc.vector.tensor_copy(dst[:, 0:512], pm[:, 0:512])
                    nc.scalar.copy(dst[:, 512:1024], pm[:, 512:1024])
                else:
                    # out evacuation fp32->fp16: ACT, except a few on DVE on
                    # the tree-free last b-tile to balance the engine load
                    dve_out = (oev[0] % 5 == 2) if flast else False
                    if dve_out:
                        nc.vector.tensor_copy(dst, pm[:])
                    else:
                        nc.scalar.copy(dst, pm[:])
                oev[0] += 1
            if flast:
                # finest stores at the drain tail: one per (gp, gqi), all
                # on the (by now empty) sync HWDGE ring; the final group
                # stores per-half so the last transfer is only 256KB
                qq0 = fhf * 32 + fgp * 16 + fgqi * 8
                if final:
                    for half in range(2):
                        nc.sync.dma_start(
                            out_d[fbt * 128:(fbt + 1) * 128,
                                  (qq0 + half * 4) * 256:(qq0 + half * 4 + 4) * 256],
                            stg[:, fgp * 4096 + fgqi * 2048 + half * 1024:
                                fgp * 4096 + fgqi * 2048 + (half + 1) * 1024],
                        )
                else:
                    nc.sync.dma_start(
                        out_d[fbt * 128:(fbt + 1) * 128,
                              qq0 * 256:(qq0 + 8) * 256],
                        stg[:, fgp * 4096 + fgqi * 2048:
                            fgp * 4096 + (fgqi + 1) * 2048],
                    )
            elif fgqi == 1:
                # per-gp stores [128,4096] (1MB) alternating rings
                store(
                    out_d[fbt * 128:(fbt + 1) * 128,
                          (fhf * 32 + fgp * 16) * 256:
                          (fhf * 32 + fgp * 16 + 16) * 256],
                    stg[:, fgp * 4096:(fgp + 1) * 4096],
                )

        for gi, grp in enumerate(groups):
            gw = len(grp)
            pl = pl0 if gi == 0 else pl1
            # group-wide arity planes [128, gw*G]
            s2 = [pl[:, i, :, :] for i in range(A)]

            tr = treep.tile([128, 20, gw, G], F16, tag="tree", name=f"tr{gi}")
            slot = [0]
            mn, mx = {}, {}

            def alloc():
                ap = tr[:, slot[0], :, :]
                slot[0] += 1
                return ap

            # (0,1) min+max first: they only need the first half-load
            mn[(0, 1)] = alloc()
            nc.vector.tensor_tensor(mn[(0, 1)], s2[0], s2[1], mybir.AluOpType.min)
            mx[(0, 1)] = alloc()
            nc.vector.tensor_tensor(mx[(0, 1)], s2[0], s2[1], mybir.AluOpType.max)
            for (i, j) in _PAIRS[1:]:
                mn[(i, j)] = alloc()
                nc.vector.tensor_tensor(mn[(i, j)], s2[i], s2[j], mybir.AluOpType.min)
            for (i, j) in _PAIRS[1:]:
                mx[(i, j)] = alloc()
                nc.vector.tensor_tensor(mx[(i, j)], s2[i], s2[j], mybir.AluOpType.max)
            for (i, j, k) in _TRIPLES:
                mn[(i, j, k)] = alloc()
                nc.vector.tensor_tensor(mn[(i, j, k)], mn[(i, j)], s2[k], mybir.AluOpType.min)
                mx[(i, j, k)] = alloc()
                nc.vector.tensor_tensor(mx[(i, j, k)], mx[(i, j)], s2[k], mybir.AluOpType.max)

            def sub_ap(S):
                return s2[S[0]] if len(S) == 1 else mn[S]

            def sup_ap(Cm):
                return s2[Cm[0]] if len(Cm) == 1 else mx[Cm]

            # one W tile per PAIR, both b-tiles side by side (t dim); the
            # bufs=1 pool reuses the same SBUF for pair1 (its writes only
            # wait on pair0's long-finished transpose reads).
            # W layout: free = t*8192 + q*128 + m*8 + gl (K-order (m,gl));
            # the walrus BIR verifier requires the transpose stationary AP
            # to have one free dim, so per-q chunks must be contiguous.
            wt = wpool.tile([128, 2 * NQ * 128], F16, tag="w", name=f"w{gi}")
            wv5 = wt.rearrange("p (t q m gl) -> p t q m gl", t=2, m=16, gl=8)

            def emit_subs(ts, hf2):
                """Emit the qmin/neg/14-sub ops for q-half ``hf2`` covering
                the b-tile slice ``ts`` of the pair: solo [128,(32,8)] ops,
                or paired [128,(2,32,8)] ops at half the op count."""
                qh = slice(hf2 * 32, hf2 * 32 + 32)
                gh = slice(hf2 * 256, hf2 * 256 + 256)
                nc.vector.tensor_tensor(
                    wv5[:, ts, qh, 15, :],
                    mn[(0, 1, 2)][:, ts, gh], s2[3][:, ts, gh],
                    mybir.AluOpType.min,
                )
                # m0 column: -qmin (x = relu(x)-relu(-x) paired with the
                # negated m=15 params rows): uniformly relu-able W
                nc.vector.tensor_scalar_mul(
                    wv5[:, ts, qh, 0, :], wv5[:, ts, qh, 15, :], -1.0
                )
                for m in _SUBS:
                    S = tuple(i for i in range(A) if (m >> i) & 1)
                    Cm = tuple(i for i in range(A) if not ((m >> i) & 1))
                    nc.vector.tensor_tensor(
                        wv5[:, ts, qh, m, :],
                        sub_ap(S)[:, ts, gh],
                        sup_ap(Cm)[:, ts, gh],
                        mybir.AluOpType.subtract,
                    )

            # which (bt2, hf) emits subs, and over which t-slice:
            #  pair0: bt0-hf0 solo (keeps the head short), hf1 paired at
            #         bt0's position, bt1-hf0 solo; pair1: fully paired at
            #         bt2's positions (bt3 rides free)
            if gi == 0:
                sub_plan = {(0, 0): slice(0, 1), (0, 1): slice(0, 2),
                            (1, 0): slice(1, 2)}
            else:
                sub_plan = {(0, 0): slice(0, 2), (0, 1): slice(0, 2)}

            for bt2, bt in enumerate(grp):
                last_bt = bt == NBT - 1
                # one-group software-pipeline skew: group g+1's transposes
                # are emitted BEFORE group g's matmuls, so the in-order PE
                # fills the W^T-evacuation latency with transpose work
                for hf in range(2):
                    if (bt2, hf) in sub_plan:
                        emit_subs(sub_plan[(bt2, hf)], hf)
                    for gp in range(2):
                        for gqi in range(2):
                            q0 = hf * 32 + gp * 16 + gqi * 8
                            if (bt, hf) not in stgs:
                                stgs[(bt, hf)] = stgp.tile(
                                    [128, 32 * 256], F16, tag="stg",
                                    name=f"stg{bt}_{hf}",
                                )
                            pt = ptp.tile([128, 8 * 128], F16, tag="pt")
                            for j in range(8):
                                q = q0 + j
                                nc.tensor.transpose(
                                    pt[:, j * 128:(j + 1) * 128],
                                    wt[:, (bt2 * NQ + q) * 128:
                                       (bt2 * NQ + q + 1) * 128],
                                    identity[:],
                                )
                            flush_group()
                            lt = lhsp.tile([128, 8 * 128], F16, tag="lt")
                            # W^T evacuation with fused relu on DVE (fp16 2x
                            # mode, ~1.7x cheaper there than on ACT)
                            nc.vector.tensor_scalar_max(lt[:], pt[:], 0.0)
                            ev[0] += 1
                            pend[0] = (bt, hf, gp, gqi, q0, lt, last_bt)
                            if ev[0] <= 2:
                                # first two groups: flush immediately (no
                                # skew) so the ACT evac stream starts ~1us
                                # earlier; the skew re-establishes after
                                flush_group()

        flush_group(final=True)

    nc.compile()
    return nc


def _get_program():
    global _cached_nc
    if _cached_nc is None:
        _cached_nc = _build_program()
    return _cached_nc


def _make_inputs(X, params):
    X = np.ascontiguousarray(X, dtype=np.float32)
    params = np.ascontiguousarray(params, dtype=np.float32)
    P4 = params.reshape(NQ, 8, 16, O)                 # [q, gl, m, o]
    # block-diag table: pbd[m*8+gl, q*256 + gl*32 + o] = params[8q+gl, m, o]
    # row m=0 carries -params[...,15,:] (pairs with the -qmin W column)
    Pb = np.zeros((16, 8, NQ, 8, O), np.float32)
    for gl in range(8):
        Pb[1:, gl, :, gl, :] = P4[:, gl, 1:, :].transpose(1, 0, 2)
        Pb[0, gl, :, gl, :] = -P4[:, gl, 15, :]
    pbd = np.ascontiguousarray(Pb.reshape(128, NQ * 256).astype(np.float16))
    # de-interleave X to per-arity fp16 planes, packing b-tile pairs:
    # xp[c, pr, p, a, t, g] = X[c*BS + pr*256 + t*128 + p, g, a]
    Xp = (X.reshape(NCORES, NBT // 2, 2, 128, G, A)
            .transpose(0, 1, 3, 5, 2, 4)              # c, pr, p, a, t, g
            .astype(np.float16))
    Xp = np.ascontiguousarray(Xp.reshape(NCORES, NPAIR * 128, A * 2 * G))
    in_maps = [
        {"x": Xp[c], "pbd": pbd}
        for c in range(NCORES)
    ]
    return in_maps


def kernel(X, params):
    nc = _get_program()
    in_maps = _make_inputs(X, params)
    res = run_bass_kernel_spmd(nc, in_maps, list(range(NCORES))).results
    out = np.concatenate(
        [res[c]["out"].astype(np.float32).reshape(BS, G, O) for c in range(NCORES)],
        axis=0,
    )
    return out


def kernel_traced(X, params):
    """Like kernel() but also returns the BassKernelResults (profile info)."""
    nc = _get_program()
    in_maps = _make_inputs(X, params)
    br = run_bass_kernel_spmd(nc, in_maps, list(range(NCORES)), trace=True)
    out = np.concatenate(
        [br.results[c]["out"].astype(np.float32).reshape(BS, G, O)
         for c in range(NCORES)],
        axis=0,
    )
    return out, br
